# revision 8
# baseline (speedup 1.0000x reference)
"""Causal single-head attention layer on 8 TRN2 NeuronCores.

Reference (per batch b):
  Q = x@Wq+bq; K = x@Wk+bk; V = x@Wv+bv        (S=4096, D=512, H=64)
  S = Q K^T / sqrt(S);  P = softmax(S + causal_mask);  out = (P V) @ Wo + bo

Sharding: 8 cores = 4 batches x 2 "halves". Each core owns 4 query-blocks
of 512 rows of its batch: even cores take blocks [7,4,3,0], odd take
[6,5,2,1] (causal work 72 k-tiles each). SPMD requires one program, so
both core types run the same *structural* schedule with per-slot k-tile
counts NKT=[32,24,16,8]; over-structural/diagonal k-tiles are killed by
adding per-core bias tiles (0 / -2048, derived on-chip from an iota ramp
and a tiny per-core threshold input) into the scores before exp, so no
mask tensors are shipped and no collectives are needed.

On-chip algorithm per core (all matmuls fp16, fp32 PSUM accumulate):
  xt (D-on-partition x^T, host-pretransposed) -> K^T,V^T proj (stacked
  [Wv|Wk] stationary) and Q^T proj on host-permuted xtq with duplicated
  [Wq|Wq] so Q^T lands on both partition halves.
  K^T is repacked (even k-tiles -> partitions 0:64, odd -> 64:128) so each
  S^T pair runs as two CONCURRENT PE row-tile matmuls (tile_position (0,0)
  and (64,0)), doubling S^T throughput.
  V^T -> V via PE transposes; V gets a ones column appended so the softmax
  denominator falls out of the AV matmul for free.
  Per group g: S^T [128k x 1024q] (+ ident@bias matmuls on masked groups)
  -> exp (ACT, scale 1/64) -> fp16 P -> AV accumulate out^T_aug [65, 512].
  Final: y = (out^T_aug.T @ [Wo; bv@Wo+bo]) * (1/denom).
  Softmax max-subtraction skipped: |S/64| <~ 1 so exp is safe.
  Slots are processed smallest-k-range first ([3,2,1,0]) so production
  stays ahead; emission is software-pipelined (AV lags S^T by 2 groups,
  projections/bias-precomputes interleaved, epilogues split in halves) so
  PE never sits behind the ACT-paced exp chain.
"""

import os
import math

os.environ.setdefault("MYCRO_LOCAL_CACHE", "1")

import numpy as np

import concourse.bass as bass
import concourse.mybir as mybir
import concourse.tile as tile
from concourse import bacc
from concourse.bass_utils import run_bass_kernel_spmd
from concourse.masks import make_identity

F32 = mybir.dt.float32
F16 = mybir.dt.float16
I16 = mybir.dt.int16

B, S, D, H = 4, 4096, 512, 64
QB = 512                  # query block
NKT = [32, 24, 16, 8]     # structural k-tiles (of 128) per slot
BLOCKS_EVEN = [7, 4, 3, 0]
BLOCKS_ODD = [6, 5, 2, 1]
NGRP = [n // 2 for n in NKT]          # groups (pairs of k-tiles) per slot
SLOT_ORDER = [3, 2, 1, 0]             # smallest k-range first
NEG_BIAS = -2048.0                    # exp(-2048/64) == 0
N_DUMMY = 9                           # PE HAM warm-up matmuls

LAST_EXEC_TIME_NS = None
LAST_RESULTS = None


def _install_ntff_hook():
    """Register the axon NTFF profile hook if the image's antenv lacks it,
    so run_bass_kernel_spmd(trace=True) can report real exec_time_ns."""
    import sys
    import types
    try:
        from antenv.axon_hooks import get_axon_ntff_profile_hook  # noqa: F401
        return True  # already present
    except ImportError:
        pass
    try:
        import trn_agent_boot.trn_boot as _tb
        hook = _tb._ntff_profile_via_ctypes("/opt/axon/libaxon_pjrt.so")
        if hook is None:
            return False
        mod = types.ModuleType("antenv.axon_hooks")
        mod.get_axon_ntff_profile_hook = lambda: hook
        mod.set_axon_ntff_profile_hook = lambda h: None
        sys.modules["antenv.axon_hooks"] = mod
        return True
    except Exception:
        return False


def _build_nc():
    nc = bacc.Bacc(
        "TRN2",
        target_bir_lowering=False,
        debug=False,
        enable_asserts=False,
        num_devices=8,
    )

    xt_d = nc.dram_tensor("xt", [D, S], F16, kind="ExternalInput")
    xtq_d = nc.dram_tensor("xtq", [D, 4 * QB], F16, kind="ExternalInput")
    wkv_d = nc.dram_tensor("wkv", [D, 128], F16, kind="ExternalInput")
    wq2_d = nc.dram_tensor("wq2", [D, 128], F16, kind="ExternalInput")
    wo_d = nc.dram_tensor("wo", [H + 1, D], F16, kind="ExternalInput")
    bkv_d = nc.dram_tensor("bkv", [128, 1], F32, kind="ExternalInput")
    bq2_d = nc.dram_tensor("bq2", [128, 1], F32, kind="ExternalInput")
    thr_d = nc.dram_tensor("thr", [128, 16], F32, kind="ExternalInput")
    out_d = nc.dram_tensor("out", [4 * QB, D], F16, kind="ExternalOutput")

    # DRAM views with the j-chunk (partition block) as an explicit dim
    xt_dv = xt_d[:, :].rearrange("(j p) c -> p j c", p=128)
    xtq_dv = xtq_d[:, :].rearrange("(j p) c -> p j c", p=128)
    wkv_dv = wkv_d[:, :].rearrange("(j p) m -> p j m", p=128)
    wq2_dv = wq2_d[:, :].rearrange("(j p) m -> p j m", p=128)

    krepeat = int(os.environ.get("KREPEAT", "1"))
    with tile.TileContext(nc) as tc:
      for _rep in range(krepeat):
        with (
            tc.tile_pool(name="big", bufs=1) as big,
            tc.tile_pool(name="small", bufs=1) as small,
        ):
            # ---- persistent SBUF tensors ----
            # xt/xtq stored "wide": the 4 j-chunks side by side so one DMA
            # fills a column block of all four chunks at once
            xtw_sb = big.tile([128, 4 * S], F16, tag="xtw")
            xtqw_sb = big.tile([128, 4 * 4 * QB], F16, tag="xtqw")
            kvt_sb = big.tile([128, S], F16, tag="kvt")     # 0:64 V^T, 64:128 K^T
            ktp_sb = big.tile([128, S // 2], F16, tag="ktp")  # packed K^T even|odd
            qtp_sb = big.tile([128, 4 * QB], F16, tag="qtp")  # Q^T dup halves
            vaug_sb = big.tile([128, 32 * 80], F16, tag="vaug")
            biasm_sb = big.tile([128, 16 * 1024], F16, tag="biasm")
            wkv_sb = small.tile([128, 4 * 128], F16, tag="wkv")
            wq_sb = small.tile([128, 4 * 128], F16, tag="wq")
            wo_sb = small.tile([H + 1, D], F16, tag="wo")
            bkv_sb = small.tile([128, 1], F32, tag="bkv")
            bq_sb = small.tile([128, 1], F32, tag="bq")
            thr_sb = small.tile([128, 16], F32, tag="thr")
            ident_sb = small.tile([64, 64], F16, tag="ident")
            id128_sb = small.tile([128, 128], F16, tag="id128")
            ones_sb = small.tile([1, 1], F16, tag="ones")
            r2i_sb = small.tile([128, 1024], I16, tag="r2i")
            r2_sb = small.tile([128, 1024], F16, tag="r2")
            dummy_sb = small.tile([64, 512], F16, tag="dummy")
            warm_sb = small.tile([1, 2], F32, tag="warm")

            xtw3 = xtw_sb[:].rearrange("p (j c) -> p j c", c=S)
            xtqw3 = xtqw_sb[:].rearrange("p (j c) -> p j c", c=4 * QB)

            # ---- input DMAs (few, large) ----
            # weights on the scalar HWDGE ring (frees scalar early for ACT)
            nc.scalar.dma_start(out=wkv_sb[:].rearrange("p (j m) -> p j m", m=128),
                                in_=wkv_dv)
            nc.scalar.dma_start(out=wq_sb[:].rearrange("p (j m) -> p j m", m=128),
                                in_=wq2_dv)
            nc.scalar.dma_start(out=bkv_sb[:], in_=bkv_d[:, :])
            nc.scalar.dma_start(out=bq_sb[:], in_=bq2_d[:, :])
            nc.scalar.dma_start(out=thr_sb[:], in_=thr_d[:, :])
            nc.scalar.dma_start(out=wo_sb[:], in_=wo_d[:, :])
            # bulk x^T on the sync HWDGE ring: first k/v column block, then
            # the q blocks in slot processing order, then the rest
            nc.sync.dma_start(out=xtw3[:, :, 0:1024], in_=xt_dv[:, :, 0:1024])
            for s in SLOT_ORDER[:2]:
                nc.sync.dma_start(
                    out=xtqw3[:, :, s * QB:(s + 1) * QB],
                    in_=xtq_dv[:, :, s * QB:(s + 1) * QB],
                )
            nc.sync.dma_start(out=xtw3[:, :, 1024:2048], in_=xt_dv[:, :, 1024:2048])
            for s in SLOT_ORDER[2:]:
                nc.sync.dma_start(
                    out=xtqw3[:, :, s * QB:(s + 1) * QB],
                    in_=xtq_dv[:, :, s * QB:(s + 1) * QB],
                )
            nc.sync.dma_start(out=xtw3[:, :, 2048:3072], in_=xt_dv[:, :, 2048:3072])
            nc.sync.dma_start(out=xtw3[:, :, 3072:4096], in_=xt_dv[:, :, 3072:4096])

            # ---- on-chip constants ----
            nc.vector.memset(dummy_sb[:], 0.0)
            nc.vector.memset(warm_sb[:, 0:1], 0.0)
            # preload the ACT exp table set before the real activations
            nc.scalar.activation(
                warm_sb[:, 1:2], warm_sb[:, 0:1],
                mybir.ActivationFunctionType.Exp,
            )
            make_identity(nc, ident_sb[:])
            make_identity(nc, id128_sb[:])
            nc.vector.memset(ones_sb[:], 1.0)
            # ramp R2[p, u*512+f] = f - p - 128*u  (for causal masking)
            nc.gpsimd.iota(
                r2i_sb[:], pattern=[[-128, 2], [1, 512]], base=0,
                channel_multiplier=-1,
            )
            nc.vector.tensor_copy(r2_sb[:], r2i_sb[:])
            vaug3 = vaug_sb[:].rearrange("p (k c) -> p k c", c=80)
            nc.vector.memset(vaug3[:, :, 64:65], 1.0)

            def emit_biasm(idx, eng):
                # bias tile: NEG_BIAS where r2 < thr[idx] (masked), else 0
                eng.tensor_scalar(
                    biasm_sb[:, idx * 1024:(idx + 1) * 1024],
                    r2_sb[:],
                    thr_sb[:, idx:idx + 1],
                    NEG_BIAS,
                    op0=mybir.AluOpType.is_lt,
                    op1=mybir.AluOpType.mult,
                )

            # slot-3 bias tiles are needed first: compute on DVE right away
            for idx in (12, 13, 14, 15):
                emit_biasm(idx, nc.vector)

            with (
                tc.tile_pool(name="projps", bufs=3, space="PSUM") as projps,
                tc.tile_pool(name="stps", bufs=2, space="PSUM") as stps,
                tc.tile_pool(name="otps", bufs=1, space="PSUM") as otps,
                tc.tile_pool(name="ptp", bufs=4) as ptp,
                tc.tile_pool(name="epi", bufs=6) as epi,
                tc.tile_pool(name="ysbp", bufs=2) as ysbp,
            ):
                # PE HAM warm-up: dummy matmuls while input DMAs stream
                for _ in range(N_DUMMY):
                    dmy = projps.tile([64, 512], F32, name="pp", tag="pp")
                    nc.tensor.matmul(
                        dmy[:], lhsT=dummy_sb[:, 0:64], rhs=dummy_sb[:],
                        start=True, stop=True,
                    )

                kv_k = kvt_sb[64:128, :].rearrange(
                    "p (g u c) -> p g u c", u=2, c=128
                )

                def emit_P(sb):
                    # KV projection for column block sb (k-tiles 4sb..4sb+3)
                    kvp = projps.tile([128, 512], F32, name="pp", tag="pp")
                    for j in range(4):
                        nc.tensor.matmul(
                            kvp[:],
                            lhsT=wkv_sb[:, j * 128:(j + 1) * 128],
                            rhs=xtw3[:, j, sb * 512:(sb + 1) * 512],
                            start=(j == 0),
                            stop=(j == 3),
                        )
                    nc.vector.tensor_scalar_add(
                        kvt_sb[:, sb * 512:(sb + 1) * 512], kvp[:], bkv_sb[:]
                    )
                    # repack K^T: even k-tiles -> partitions 0:64, odd -> 64:128
                    nc.gpsimd.dma_start(
                        out=ktp_sb[0:64, sb * 256:(sb + 1) * 256],
                        in_=kv_k[:, 2 * sb:2 * sb + 2, 0:1, :],
                    )
                    nc.gpsimd.dma_start(
                        out=ktp_sb[64:128, sb * 256:(sb + 1) * 256],
                        in_=kv_k[:, 2 * sb:2 * sb + 2, 1:2, :],
                    )
                    # V natural tiles via PE transpose
                    for kt in range(4 * sb, 4 * sb + 4):
                        vtp = projps.tile([128, 64], F16, name="pp", tag="pp")
                        nc.tensor.transpose(
                            vtp[:], kvt_sb[0:64, kt * 128:(kt + 1) * 128],
                            ident_sb[:],
                        )
                        nc.vector.tensor_copy(
                            vaug_sb[:, kt * 80:kt * 80 + 64], vtp[:]
                        )

                def emit_Q(s):
                    qp = projps.tile([128, 512], F32, name="pp", tag="pp")
                    for j in range(4):
                        nc.tensor.matmul(
                            qp[:],
                            lhsT=wq_sb[:, j * 128:(j + 1) * 128],
                            rhs=xtqw3[:, j, s * 512:(s + 1) * 512],
                            start=(j == 0),
                            stop=(j == 3),
                        )
                    nc.vector.tensor_scalar_add(
                        qtp_sb[:, s * 512:(s + 1) * 512], qp[:], bq_sb[:]
                    )

                groups = [(s, g) for s in SLOT_ORDER for g in range(NGRP[s])]
                otp_of = {}
                pt_of = {}

                def emit_S(i):
                    s, g = groups[i]
                    if g == 0:
                        otp_of[s] = otps.tile(
                            [H + 1, 512], F32, name="otp", tag="otp"
                        )
                    masked = g >= NGRP[s] - 4
                    stp = stps.tile([128, 1024], F32, name="stp", tag="stp")
                    nc.tensor.matmul(
                        stp[:, 0:512],
                        lhsT=ktp_sb[0:64, g * 128:(g + 1) * 128],
                        rhs=qtp_sb[0:64, s * 512:(s + 1) * 512],
                        start=True, stop=not masked,
                        tile_position=(0, 0),
                    )
                    nc.tensor.matmul(
                        stp[:, 512:1024],
                        lhsT=ktp_sb[64:128, g * 128:(g + 1) * 128],
                        rhs=qtp_sb[64:128, s * 512:(s + 1) * 512],
                        start=True, stop=not masked,
                        tile_position=(64, 0),
                    )
                    if masked:
                        # add 0/-2048 bias tiles into the scores on the PE
                        idx = s * 4 + (g - (NGRP[s] - 4))
                        for u in range(2):
                            nc.tensor.matmul(
                                stp[:, u * 512:(u + 1) * 512],
                                lhsT=id128_sb[:],
                                rhs=biasm_sb[:, idx * 1024 + u * 512:
                                             idx * 1024 + (u + 1) * 512],
                                start=False, stop=True,
                            )
                    pt = ptp.tile([128, 1024], F16, name="pt", tag="pt")
                    nc.scalar.activation(
                        pt[:], stp[:], mybir.ActivationFunctionType.Exp,
                        scale=1.0 / 64.0,
                    )
                    pt_of[i] = pt

                def emit_AV(i):
                    s, g = groups[i]
                    pt = pt_of.pop(i)
                    for u in range(2):
                        kt = 2 * g + u
                        nc.tensor.matmul(
                            otp_of[s][:],
                            lhsT=vaug_sb[:, kt * 80:kt * 80 + 65],
                            rhs=pt[:, u * 512:(u + 1) * 512],
                            start=(kt == 0),
                            stop=(kt == NKT[s] - 1),
                        )

                epi_st = {}

                def emit_E_half(s, half):
                    if half == 0:
                        otp = otp_of.pop(s)
                        ot16 = epi.tile([H + 1, 512], F16, name="ot16", tag="ot16")
                        dnrow = epi.tile([1, 512], F16, name="dnrow", tag="dnrow")
                        nc.vector.tensor_copy(ot16[:], otp[:])
                        nc.vector.tensor_copy(dnrow[:], otp[64:65, :])
                        epi_st[s] = (ot16, dnrow)
                    ot16, dnrow = epi_st[s]
                    for t in (0, 1) if half == 0 else (2, 3):
                        dnp = projps.tile([128, 1], F32, name="pp", tag="pp")
                        nc.tensor.matmul(
                            dnp[:],
                            lhsT=dnrow[:, t * 128:(t + 1) * 128],
                            rhs=ones_sb[:],
                            start=True, stop=True,
                        )
                        recip = epi.tile([128, 1], F32, name="recip", tag="recip")
                        nc.vector.reciprocal(recip[:], dnp[:])
                        yp = projps.tile([128, 512], F32, name="pp", tag="pp")
                        nc.tensor.matmul(
                            yp[:],
                            lhsT=ot16[:, t * 128:(t + 1) * 128],
                            rhs=wo_sb[:],
                            start=True, stop=True,
                        )
                        ysb = ysbp.tile([128, 512], F16, name="ysb", tag="ysb")
                        nc.vector.tensor_scalar_mul(ysb[:], yp[:], recip[:])
                        nc.sync.dma_start(
                            out=out_d[s * 512 + t * 128: s * 512 + (t + 1) * 128, :],
                            in_=ysb[:],
                        )

                # ---- software-pipelined emission ----
                gb = emit_biasm
                prod = {
                    0: [lambda: emit_P(0), lambda: emit_Q(3),
                        lambda: emit_P(1), lambda: emit_Q(2)],
                    4: [lambda: emit_P(2), lambda: gb(8, nc.vector),
                        lambda: gb(9, nc.vector)],
                    6: [lambda: gb(10, nc.gpsimd), lambda: gb(11, nc.gpsimd)],
                    8: [lambda: emit_P(3)],
                    12: [lambda: emit_Q(1), lambda: emit_P(4)],
                    14: [lambda: gb(4, nc.gpsimd), lambda: gb(5, nc.gpsimd)],
                    16: [lambda: emit_P(5), lambda: gb(6, nc.gpsimd),
                         lambda: gb(7, nc.gpsimd)],
                    20: [lambda: emit_P(6)],
                    24: [lambda: emit_Q(0), lambda: gb(0, nc.gpsimd),
                         lambda: gb(1, nc.gpsimd)],
                    28: [lambda: emit_P(7), lambda: gb(2, nc.gpsimd),
                         lambda: gb(3, nc.gpsimd)],
                }
                last_step_of_slot = {}
                acc = -1
                for s in SLOT_ORDER:
                    acc += NGRP[s]
                    last_step_of_slot[s] = acc

                n = len(groups)
                for i in range(n + 3):
                    for fn in prod.get(i, []):
                        fn()
                    if i < n:
                        emit_S(i)
                    if 0 <= i - 2 < n:
                        emit_AV(i - 2)
                        for s in SLOT_ORDER:
                            if last_step_of_slot[s] == i - 2:
                                emit_E_half(s, 0)
                    if 0 <= i - 3 < n:
                        for s in SLOT_ORDER:
                            if last_step_of_slot[s] == i - 3:
                                emit_E_half(s, 1)

    nc.compile()
    return nc


_NC_CACHE = {}


def _thresholds(blocks):
    # mask P[k_local, u*512+f] iff  f - p - 128*u < thr[s, j]
    # thr = 128*t0 - 512*block  with t0 = NKT[s]-8+2j  (even tile of group)
    t = np.zeros(16, np.float32)
    for s in range(4):
        for j in range(4):
            t0 = NKT[s] - 8 + 2 * j
            t[s * 4 + j] = 128.0 * t0 - 512.0 * blocks[s]
    return np.tile(t[None, :], (128, 1)).astype(np.float32)


def _make_in_maps(x, Wq, bq, Wk, bk, Wv, bv, Wo, bo):
    wkv = np.concatenate([Wv, Wk], axis=1).astype(np.float16)     # (512, 128)
    wq2 = np.concatenate([Wq, Wq], axis=1).astype(np.float16)     # (512, 128)
    bkv = np.concatenate([np.zeros(64, np.float32), bk])[:, None].astype(np.float32)
    bq2 = np.concatenate([bq, bq])[:, None].astype(np.float32)
    wo_aug = np.concatenate(
        [Wo, (bv @ Wo + bo)[None, :]], axis=0
    ).astype(np.float16)
    thr_even = _thresholds(BLOCKS_EVEN)
    thr_odd = _thresholds(BLOCKS_ODD)

    in_maps = []
    for c in range(8):
        b = c // 2
        blocks = BLOCKS_EVEN if c % 2 == 0 else BLOCKS_ODD
        xt = np.ascontiguousarray(x[b].T).astype(np.float16)      # (512, 4096)
        qcols = np.concatenate(
            [np.arange(blk * QB, (blk + 1) * QB) for blk in blocks]
        )
        xtq = np.ascontiguousarray(xt[:, qcols])                  # (512, 2048)
        in_maps.append({
            "xt": xt,
            "xtq": xtq,
            "wkv": wkv,
            "wq2": wq2,
            "wo": wo_aug,
            "bkv": bkv,
            "bq2": bq2,
            "thr": thr_even if c % 2 == 0 else thr_odd,
        })
    return in_maps


def kernel(x, Wq, bq, Wk, bk, Wv, bv, Wo, bo):
    global LAST_EXEC_TIME_NS, LAST_RESULTS
    x = np.asarray(x, dtype=np.float32)
    Wq, bq = np.asarray(Wq, np.float32), np.asarray(bq, np.float32)
    Wk, bk = np.asarray(Wk, np.float32), np.asarray(bk, np.float32)
    Wv, bv = np.asarray(Wv, np.float32), np.asarray(bv, np.float32)
    Wo, bo = np.asarray(Wo, np.float32), np.asarray(bo, np.float32)

    if "nc" not in _NC_CACHE:
        _NC_CACHE["nc"] = _build_nc()
    nc = _NC_CACHE["nc"]

    in_maps = _make_in_maps(x, Wq, bq, Wk, bk, Wv, bv, Wo, bo)

    trace = os.environ.get("KERNEL_TRACE", "1") == "1"
    if trace:
        trace = _install_ntff_hook()
    tmpdir = os.environ.get("KERNEL_TRACE_DIR") or None
    try:
        res = run_bass_kernel_spmd(
            nc, in_maps, core_ids=list(range(8)), trace=trace, tmpdir=tmpdir
        )
    except Exception:
        if not trace:
            raise
        res = run_bass_kernel_spmd(nc, in_maps, core_ids=list(range(8)), trace=False)
    LAST_EXEC_TIME_NS = res.exec_time_ns
    LAST_RESULTS = res

    out = np.empty((B, S, D), np.float32)
    for c in range(8):
        b = c // 2
        blocks = BLOCKS_EVEN if c % 2 == 0 else BLOCKS_ODD
        shard = np.asarray(res.results[c]["out"], dtype=np.float32)
        for sidx, blk in enumerate(blocks):
            out[b, blk * QB:(blk + 1) * QB, :] = shard[sidx * QB:(sidx + 1) * QB, :]
    return out


# revision 9
# speedup vs baseline: 2.1566x; 2.1566x over previous
"""Causal single-head attention layer on 8 TRN2 NeuronCores.

Reference (per batch b):
  Q = x@Wq+bq; K = x@Wk+bk; V = x@Wv+bv        (S=4096, D=512, H=64)
  S = Q K^T / sqrt(S);  P = softmax(S + causal_mask);  out = (P V) @ Wo + bo

Sharding: 8 cores = 4 batches x 2 "halves". Each core owns 4 query-blocks
of 512 rows of its batch: even cores take blocks [7,4,3,0], odd take
[6,5,2,1] (causal work 72 k-tiles each). SPMD requires one program, so
both core types run the same *structural* schedule with per-slot k-tile
counts NKT=[32,24,16,8]; over-structural/diagonal k-tiles are killed by
adding per-core bias tiles (0 / -2048, derived on-chip from an iota ramp
and a tiny per-core threshold input) into the scores before exp, so no
mask tensors are shipped and no collectives are needed.

On-chip algorithm per core (all matmuls fp16, fp32 PSUM accumulate):
  xt (D-on-partition x^T, host-pretransposed) -> K^T,V^T proj (stacked
  [Wv|Wk] stationary) and Q^T proj on host-permuted xtq with duplicated
  [Wq|Wq] so Q^T lands on both partition halves.
  K^T is repacked (even k-tiles -> partitions 0:64, odd -> 64:128) so each
  S^T pair runs as two CONCURRENT PE row-tile matmuls (tile_position (0,0)
  and (64,0)), doubling S^T throughput.
  V^T -> V via PE transposes; V gets a ones column appended so the softmax
  denominator falls out of the AV matmul for free.
  Per group g: S^T [128k x 1024q] (+ ident@bias matmuls on masked groups)
  -> exp (ACT, scale 1/64) -> fp16 P -> AV accumulate out^T_aug [65, 512].
  Final: y = (out^T_aug.T @ [Wo; bv@Wo+bo]) * (1/denom).
  Softmax max-subtraction skipped: |S/64| <~ 1 so exp is safe.
  Slots are processed smallest-k-range first ([3,2,1,0]) so production
  stays ahead; emission is software-pipelined (AV lags S^T by 2 groups,
  projections/bias-precomputes interleaved, epilogues split in halves) so
  PE never sits behind the ACT-paced exp chain.
"""

import os
import math

os.environ.setdefault("MYCRO_LOCAL_CACHE", "1")

import numpy as np

import concourse.bass as bass
import concourse.mybir as mybir
import concourse.tile as tile
from concourse import bacc
from concourse.bass_utils import run_bass_kernel_spmd
from concourse.masks import make_identity

F32 = mybir.dt.float32
F16 = mybir.dt.float16
I16 = mybir.dt.int16

B, S, D, H = 4, 4096, 512, 64
QB = 512                  # query block
NKT = [32, 24, 16, 8]     # structural k-tiles (of 128) per slot
BLOCKS_EVEN = [7, 4, 3, 0]
BLOCKS_ODD = [6, 5, 2, 1]
NGRP = [n // 2 for n in NKT]          # groups (pairs of k-tiles) per slot
SLOT_ORDER = [3, 2, 1, 0]             # smallest k-range first
NEG_BIAS = -2048.0                    # exp(-2048/64) == 0
N_DUMMY = 9                           # PE HAM warm-up matmuls

LAST_EXEC_TIME_NS = None
LAST_RESULTS = None


def _install_ntff_hook():
    """Register the axon NTFF profile hook if the image's antenv lacks it,
    so run_bass_kernel_spmd(trace=True) can report real exec_time_ns."""
    import sys
    import types
    try:
        from antenv.axon_hooks import get_axon_ntff_profile_hook  # noqa: F401
        return True  # already present
    except ImportError:
        pass
    try:
        import trn_agent_boot.trn_boot as _tb
        hook = _tb._ntff_profile_via_ctypes("/opt/axon/libaxon_pjrt.so")
        if hook is None:
            return False
        mod = types.ModuleType("antenv.axon_hooks")
        mod.get_axon_ntff_profile_hook = lambda: hook
        mod.set_axon_ntff_profile_hook = lambda h: None
        sys.modules["antenv.axon_hooks"] = mod
        return True
    except Exception:
        return False


def _build_nc():
    nc = bacc.Bacc(
        "TRN2",
        target_bir_lowering=False,
        debug=False,
        enable_asserts=False,
        num_devices=8,
    )

    xt_d = nc.dram_tensor("xt", [D, S], F16, kind="ExternalInput")
    xtq_d = nc.dram_tensor("xtq", [D, 4 * QB], F16, kind="ExternalInput")
    wkv_d = nc.dram_tensor("wkv", [D, 128], F16, kind="ExternalInput")
    wq2_d = nc.dram_tensor("wq2", [D, 128], F16, kind="ExternalInput")
    wo_d = nc.dram_tensor("wo", [H + 1, D], F16, kind="ExternalInput")
    bkv_d = nc.dram_tensor("bkv", [128, 1], F32, kind="ExternalInput")
    bq2_d = nc.dram_tensor("bq2", [128, 1], F32, kind="ExternalInput")
    biasm_d = nc.dram_tensor("biasm", [128, 16 * 1024], F16, kind="ExternalInput")
    out_d = nc.dram_tensor("out", [4 * QB, D], F16, kind="ExternalOutput")

    # DRAM views with the j-chunk (partition block) as an explicit dim
    xt_dv = xt_d[:, :].rearrange("(j p) c -> p j c", p=128)
    xtq_dv = xtq_d[:, :].rearrange("(j p) c -> p j c", p=128)
    wkv_dv = wkv_d[:, :].rearrange("(j p) m -> p j m", p=128)
    wq2_dv = wq2_d[:, :].rearrange("(j p) m -> p j m", p=128)

    krepeat = int(os.environ.get("KREPEAT", "1"))
    with tile.TileContext(nc) as tc:
      for _rep in range(krepeat):
        with (
            tc.tile_pool(name="big", bufs=1) as big,
            tc.tile_pool(name="small", bufs=1) as small,
        ):
            # ---- persistent SBUF tensors ----
            # xt/xtq stored "wide": the 4 j-chunks side by side so one DMA
            # fills a column block of all four chunks at once
            xtw_sb = big.tile([128, 4 * S], F16, tag="xtw")
            xtqw_sb = big.tile([128, 4 * 4 * QB], F16, tag="xtqw")
            kvt_sb = big.tile([128, S], F16, tag="kvt")     # 0:64 V^T, 64:128 K^T
            ktp_sb = big.tile([128, S // 2], F16, tag="ktp")  # packed K^T even|odd
            qtp_sb = big.tile([128, 4 * QB], F16, tag="qtp")  # Q^T dup halves
            vaug_sb = big.tile([128, 32 * 80], F16, tag="vaug")
            biasm_sb = big.tile([128, 16 * 1024], F16, tag="biasm")
            wkv_sb = small.tile([128, 4 * 128], F16, tag="wkv")
            wq_sb = small.tile([128, 4 * 128], F16, tag="wq")
            wo_sb = small.tile([H + 1, D], F16, tag="wo")
            bkv_sb = small.tile([128, 1], F32, tag="bkv")
            bq_sb = small.tile([128, 1], F32, tag="bq")
            ident_sb = small.tile([64, 64], F16, tag="ident")
            id128_sb = small.tile([128, 128], F16, tag="id128")
            ones_sb = small.tile([1, 1], F16, tag="ones")
            dummy_sb = small.tile([64, 512], F16, tag="dummy")
            warm_sb = small.tile([1, 2], F32, tag="warm")

            xtw3 = xtw_sb[:].rearrange("p (j c) -> p j c", c=S)
            xtqw3 = xtqw_sb[:].rearrange("p (j c) -> p j c", c=4 * QB)

            # ---- input DMAs (few, large) ----
            # weights on the scalar HWDGE ring (frees scalar early for ACT)
            nc.scalar.dma_start(out=wkv_sb[:].rearrange("p (j m) -> p j m", m=128),
                                in_=wkv_dv)
            nc.scalar.dma_start(out=wq_sb[:].rearrange("p (j m) -> p j m", m=128),
                                in_=wq2_dv)
            nc.scalar.dma_start(out=bkv_sb[:], in_=bkv_d[:, :])
            nc.scalar.dma_start(out=bq_sb[:], in_=bq2_d[:, :])
            nc.scalar.dma_start(out=wo_sb[:], in_=wo_d[:, :])
            # bulk x^T on the sync HWDGE ring: first k/v column block, then
            # the q blocks in slot processing order, then the rest
            nc.sync.dma_start(out=xtw3[:, :, 0:1024], in_=xt_dv[:, :, 0:1024])
            for s in SLOT_ORDER[:2]:
                nc.sync.dma_start(
                    out=xtqw3[:, :, s * QB:(s + 1) * QB],
                    in_=xtq_dv[:, :, s * QB:(s + 1) * QB],
                )
            nc.sync.dma_start(out=xtw3[:, :, 1024:2048], in_=xt_dv[:, :, 1024:2048])
            for s in SLOT_ORDER[2:]:
                nc.sync.dma_start(
                    out=xtqw3[:, :, s * QB:(s + 1) * QB],
                    in_=xtq_dv[:, :, s * QB:(s + 1) * QB],
                )
            nc.sync.dma_start(out=xtw3[:, :, 2048:3072], in_=xt_dv[:, :, 2048:3072])
            nc.sync.dma_start(out=xtw3[:, :, 3072:4096], in_=xt_dv[:, :, 3072:4096])

            # ---- on-chip constants ----
            nc.vector.memset(dummy_sb[:], 0.0)
            nc.vector.memset(warm_sb[:, 0:1], 0.0)
            # preload the ACT exp table set before the real activations
            nc.scalar.activation(
                warm_sb[:, 1:2], warm_sb[:, 0:1],
                mybir.ActivationFunctionType.Exp,
            )
            make_identity(nc, ident_sb[:])
            make_identity(nc, id128_sb[:])
            nc.vector.memset(ones_sb[:], 1.0)
            vaug3 = vaug_sb[:].rearrange("p (k c) -> p k c", c=80)
            nc.vector.memset(vaug3[:, :, 64:65], 1.0)

            def emit_biasm_dma(idx0):
                # ship 4 precomputed 0/-2048 bias tiles on the gpsimd ring
                nc.gpsimd.dma_start(
                    out=biasm_sb[:, idx0 * 1024:(idx0 + 4) * 1024],
                    in_=biasm_d[:, idx0 * 1024:(idx0 + 4) * 1024],
                )

            # slot-3 bias tiles are needed first
            emit_biasm_dma(12)

            with (
                tc.tile_pool(name="projps", bufs=3, space="PSUM") as projps,
                tc.tile_pool(name="stps", bufs=2, space="PSUM") as stps,
                tc.tile_pool(name="otps", bufs=1, space="PSUM") as otps,
                tc.tile_pool(name="ptp", bufs=4) as ptp,
                tc.tile_pool(name="epi", bufs=6) as epi,
                tc.tile_pool(name="ysbp", bufs=2) as ysbp,
            ):
                # PE HAM warm-up: dummy matmuls while input DMAs stream
                for _ in range(N_DUMMY):
                    dmy = projps.tile([64, 512], F32, name="pp", tag="pp")
                    nc.tensor.matmul(
                        dmy[:], lhsT=dummy_sb[:, 0:64], rhs=dummy_sb[:],
                        start=True, stop=True,
                    )

                kv_k = kvt_sb[64:128, :].rearrange(
                    "p (g u c) -> p g u c", u=2, c=128
                )

                def emit_P(sb):
                    # KV projection for column block sb (k-tiles 4sb..4sb+3)
                    kvp = projps.tile([128, 512], F32, name="pp", tag="pp")
                    for j in range(4):
                        nc.tensor.matmul(
                            kvp[:],
                            lhsT=wkv_sb[:, j * 128:(j + 1) * 128],
                            rhs=xtw3[:, j, sb * 512:(sb + 1) * 512],
                            start=(j == 0),
                            stop=(j == 3),
                        )
                    nc.vector.tensor_scalar_add(
                        kvt_sb[:, sb * 512:(sb + 1) * 512], kvp[:], bkv_sb[:]
                    )
                    # repack K^T: even k-tiles -> partitions 0:64, odd -> 64:128
                    nc.gpsimd.dma_start(
                        out=ktp_sb[0:64, sb * 256:(sb + 1) * 256],
                        in_=kv_k[:, 2 * sb:2 * sb + 2, 0:1, :],
                    )
                    nc.gpsimd.dma_start(
                        out=ktp_sb[64:128, sb * 256:(sb + 1) * 256],
                        in_=kv_k[:, 2 * sb:2 * sb + 2, 1:2, :],
                    )
                    # V natural tiles via PE transpose
                    for kt in range(4 * sb, 4 * sb + 4):
                        vtp = projps.tile([128, 64], F16, name="pp", tag="pp")
                        nc.tensor.transpose(
                            vtp[:], kvt_sb[0:64, kt * 128:(kt + 1) * 128],
                            ident_sb[:],
                        )
                        nc.vector.tensor_copy(
                            vaug_sb[:, kt * 80:kt * 80 + 64], vtp[:]
                        )

                def emit_Q(s):
                    qp = projps.tile([128, 512], F32, name="pp", tag="pp")
                    for j in range(4):
                        nc.tensor.matmul(
                            qp[:],
                            lhsT=wq_sb[:, j * 128:(j + 1) * 128],
                            rhs=xtqw3[:, j, s * 512:(s + 1) * 512],
                            start=(j == 0),
                            stop=(j == 3),
                        )
                    nc.vector.tensor_scalar_add(
                        qtp_sb[:, s * 512:(s + 1) * 512], qp[:], bq_sb[:]
                    )

                groups = [(s, g) for s in SLOT_ORDER for g in range(NGRP[s])]
                otp_of = {}
                pt_of = {}

                def emit_S(i):
                    s, g = groups[i]
                    if g == 0:
                        otp_of[s] = otps.tile(
                            [H + 1, 512], F32, name="otp", tag="otp"
                        )
                    masked = g >= NGRP[s] - 4
                    stp = stps.tile([128, 1024], F32, name="stp", tag="stp")
                    nc.tensor.matmul(
                        stp[:, 0:512],
                        lhsT=ktp_sb[0:64, g * 128:(g + 1) * 128],
                        rhs=qtp_sb[0:64, s * 512:(s + 1) * 512],
                        start=True, stop=not masked,
                        tile_position=(0, 0),
                    )
                    nc.tensor.matmul(
                        stp[:, 512:1024],
                        lhsT=ktp_sb[64:128, g * 128:(g + 1) * 128],
                        rhs=qtp_sb[64:128, s * 512:(s + 1) * 512],
                        start=True, stop=not masked,
                        tile_position=(64, 0),
                    )
                    if masked:
                        # add 0/-2048 bias tiles into the scores on the PE
                        idx = s * 4 + (g - (NGRP[s] - 4))
                        for u in range(2):
                            nc.tensor.matmul(
                                stp[:, u * 512:(u + 1) * 512],
                                lhsT=id128_sb[:],
                                rhs=biasm_sb[:, idx * 1024 + u * 512:
                                             idx * 1024 + (u + 1) * 512],
                                start=False, stop=True,
                            )
                    pt = ptp.tile([128, 1024], F16, name="pt", tag="pt")
                    nc.scalar.activation(
                        pt[:], stp[:], mybir.ActivationFunctionType.Exp,
                        scale=1.0 / 64.0,
                    )
                    pt_of[i] = pt

                def emit_AV(i):
                    s, g = groups[i]
                    pt = pt_of.pop(i)
                    for u in range(2):
                        kt = 2 * g + u
                        nc.tensor.matmul(
                            otp_of[s][:],
                            lhsT=vaug_sb[:, kt * 80:kt * 80 + 65],
                            rhs=pt[:, u * 512:(u + 1) * 512],
                            start=(kt == 0),
                            stop=(kt == NKT[s] - 1),
                        )

                epi_st = {}

                def emit_E_half(s, half):
                    if half == 0:
                        otp = otp_of.pop(s)
                        ot16 = epi.tile([H + 1, 512], F16, name="ot16", tag="ot16")
                        dnrow = epi.tile([1, 512], F16, name="dnrow", tag="dnrow")
                        nc.vector.tensor_copy(ot16[:], otp[:])
                        nc.vector.tensor_copy(dnrow[:], otp[64:65, :])
                        epi_st[s] = (ot16, dnrow)
                    ot16, dnrow = epi_st[s]
                    for t in (0, 1) if half == 0 else (2, 3):
                        dnp = projps.tile([128, 1], F32, name="pp", tag="pp")
                        nc.tensor.matmul(
                            dnp[:],
                            lhsT=dnrow[:, t * 128:(t + 1) * 128],
                            rhs=ones_sb[:],
                            start=True, stop=True,
                        )
                        recip = epi.tile([128, 1], F32, name="recip", tag="recip")
                        nc.vector.reciprocal(recip[:], dnp[:])
                        yp = projps.tile([128, 512], F32, name="pp", tag="pp")
                        nc.tensor.matmul(
                            yp[:],
                            lhsT=ot16[:, t * 128:(t + 1) * 128],
                            rhs=wo_sb[:],
                            start=True, stop=True,
                        )
                        ysb = ysbp.tile([128, 512], F16, name="ysb", tag="ysb")
                        nc.vector.tensor_scalar_mul(ysb[:], yp[:], recip[:])
                        nc.sync.dma_start(
                            out=out_d[s * 512 + t * 128: s * 512 + (t + 1) * 128, :],
                            in_=ysb[:],
                        )

                # ---- software-pipelined emission ----
                prod = {
                    0: [lambda: emit_P(0), lambda: emit_Q(3),
                        lambda: emit_P(1), lambda: emit_Q(2)],
                    4: [lambda: emit_P(2), lambda: emit_biasm_dma(8)],
                    8: [lambda: emit_P(3)],
                    12: [lambda: emit_Q(1), lambda: emit_P(4),
                         lambda: emit_biasm_dma(4)],
                    16: [lambda: emit_P(5)],
                    20: [lambda: emit_P(6)],
                    24: [lambda: emit_Q(0), lambda: emit_biasm_dma(0)],
                    28: [lambda: emit_P(7)],
                }
                last_step_of_slot = {}
                acc = -1
                for s in SLOT_ORDER:
                    acc += NGRP[s]
                    last_step_of_slot[s] = acc

                n = len(groups)
                for i in range(n + 3):
                    for fn in prod.get(i, []):
                        fn()
                    if i < n:
                        emit_S(i)
                    if 0 <= i - 2 < n:
                        emit_AV(i - 2)
                        for s in SLOT_ORDER:
                            if last_step_of_slot[s] == i - 2:
                                emit_E_half(s, 0)
                    if 0 <= i - 3 < n:
                        for s in SLOT_ORDER:
                            if last_step_of_slot[s] == i - 3:
                                emit_E_half(s, 1)

    nc.compile()
    return nc


_NC_CACHE = {}


def _bias_tiles(blocks):
    # bias[p, idx, u*512+f] = NEG_BIAS where masked: f - p - 128*u < thr
    # thr = 128*t0 - 512*block  with t0 = NKT[s]-8+2j  (even tile of group)
    p = np.arange(128)[:, None, None]
    cols = np.arange(1024)[None, None, :]
    r2 = (cols % 512) - p - 128 * (cols // 512)
    thr = np.zeros((1, 16, 1), np.float32)
    for s in range(4):
        for j in range(4):
            t0 = NKT[s] - 8 + 2 * j
            thr[0, s * 4 + j, 0] = 128.0 * t0 - 512.0 * blocks[s]
    bias = np.where(r2 < thr, np.float32(NEG_BIAS), np.float32(0.0))
    return bias.reshape(128, 16 * 1024).astype(np.float16)


def _make_in_maps(x, Wq, bq, Wk, bk, Wv, bv, Wo, bo):
    wkv = np.concatenate([Wv, Wk], axis=1).astype(np.float16)     # (512, 128)
    wq2 = np.concatenate([Wq, Wq], axis=1).astype(np.float16)     # (512, 128)
    bkv = np.concatenate([np.zeros(64, np.float32), bk])[:, None].astype(np.float32)
    bq2 = np.concatenate([bq, bq])[:, None].astype(np.float32)
    wo_aug = np.concatenate(
        [Wo, (bv @ Wo + bo)[None, :]], axis=0
    ).astype(np.float16)
    biasm_even = _bias_tiles(BLOCKS_EVEN)
    biasm_odd = _bias_tiles(BLOCKS_ODD)

    in_maps = []
    for c in range(8):
        b = c // 2
        blocks = BLOCKS_EVEN if c % 2 == 0 else BLOCKS_ODD
        xt = np.ascontiguousarray(x[b].T).astype(np.float16)      # (512, 4096)
        qcols = np.concatenate(
            [np.arange(blk * QB, (blk + 1) * QB) for blk in blocks]
        )
        xtq = np.ascontiguousarray(xt[:, qcols])                  # (512, 2048)
        in_maps.append({
            "xt": xt,
            "xtq": xtq,
            "wkv": wkv,
            "wq2": wq2,
            "wo": wo_aug,
            "bkv": bkv,
            "bq2": bq2,
            "biasm": biasm_even if c % 2 == 0 else biasm_odd,
        })
    return in_maps


def kernel(x, Wq, bq, Wk, bk, Wv, bv, Wo, bo):
    global LAST_EXEC_TIME_NS, LAST_RESULTS
    x = np.asarray(x, dtype=np.float32)
    Wq, bq = np.asarray(Wq, np.float32), np.asarray(bq, np.float32)
    Wk, bk = np.asarray(Wk, np.float32), np.asarray(bk, np.float32)
    Wv, bv = np.asarray(Wv, np.float32), np.asarray(bv, np.float32)
    Wo, bo = np.asarray(Wo, np.float32), np.asarray(bo, np.float32)

    if "nc" not in _NC_CACHE:
        _NC_CACHE["nc"] = _build_nc()
    nc = _NC_CACHE["nc"]

    in_maps = _make_in_maps(x, Wq, bq, Wk, bk, Wv, bv, Wo, bo)

    trace = os.environ.get("KERNEL_TRACE", "1") == "1"
    if trace:
        trace = _install_ntff_hook()
    tmpdir = os.environ.get("KERNEL_TRACE_DIR") or None
    try:
        res = run_bass_kernel_spmd(
            nc, in_maps, core_ids=list(range(8)), trace=trace, tmpdir=tmpdir
        )
    except Exception:
        if not trace:
            raise
        res = run_bass_kernel_spmd(nc, in_maps, core_ids=list(range(8)), trace=False)
    LAST_EXEC_TIME_NS = res.exec_time_ns
    LAST_RESULTS = res

    out = np.empty((B, S, D), np.float32)
    for c in range(8):
        b = c // 2
        blocks = BLOCKS_EVEN if c % 2 == 0 else BLOCKS_ODD
        shard = np.asarray(res.results[c]["out"], dtype=np.float32)
        for sidx, blk in enumerate(blocks):
            out[b, blk * QB:(blk + 1) * QB, :] = shard[sidx * QB:(sidx + 1) * QB, :]
    return out
